# revision 12
# baseline (speedup 1.0000x reference)
"""AdaptiveGraphPooling Trainium2 kernel (8 NeuronCores, SPMD), v2.

Strategy: 256 whole graphs per core (nodes sorted by graph id). Two DRAM
copies of x per core, both loaded once (the kernel is HBM-bound, ~59MB/core):

  fm: xT [128, GPC*LFIX] bf16 feature-major, slots zero-padded to LFIX.
      Feeds the attention-score path and the max pool.
  nm: xs [128, NW*128] fp8e4m3 node-major "staged" layout (slots padded to
      LNM = 128-aligned; window w = 128 nodes; xs[p, w*128+d] =
      x_nm[w*128+p, d]) so the DMA is contiguous per partition. Feeds the
      mean-sum and attention-weighted-sum pools as PE matmuls.

Per supergroup of 32 slots:
  - h^T = attW.T @ x: column-tiled matmul pairs (M=64 at psum partitions
    0:64 / 64:128 run concurrently on different PE column groups).
  - tanh on [128, 1024] psum spans -> thc bf16 (2 nodes per column).
  - scores: one-hot ctx lhsT matmuls, 4x column-tiled by subgroup (M=32 at
    tile_position (0, 32c)); full-K lhsT with the inactive half zeroed.
    Static "runs" map each slot's nodes to thc columns.
  - exp on the compact [128, LFIX] score rows -> E_sb bf16 + z accum.
  - e rearrange to window-partition layout: DMA subgroup rows to partition
    base 0, PE-transpose [8, 128] blocks -> psum bf16 -> one DVE evac into
    rhs_t fp8 (ones block | e block).
  - pools: one fp8 matmul per 128-node window (lhsT = window, rhs =
    (ones, e) strided pair) accumulating (sum, exsum) per slot into a
    resident psum bank [128, 512].
  - max pool: DVE tensor_scalar (4x mode) accum per slot from fm.
Final: per 128-slot block, normalize, transpose, selector MLP, softmax mix.
Host concatenates per-core outputs -> [2048, 128] f32. No collectives; no
data-dependent constants are baked into the instruction stream (LFIX is
raised in 64-steps and the kernel rebuilt iff an input graph exceeds it).
"""

import math
import numpy as np
import ml_dtypes

from concourse import bass, bacc, mybir
from concourse import tile as tile_mod
from concourse.bass_utils import run_bass_kernel_spmd

BF16 = mybir.dt.bfloat16
F32 = mybir.dt.float32
FP8 = mybir.dt.float8e4
FP8NP = ml_dtypes.float8_e4m3
ALU = mybir.AluOpType
ACTF = mybir.ActivationFunctionType

D = 128
A = 64
NCORES = 8
LFIX0 = 576               # default slot width (fm); raised if a graph is larger
SG = 32                   # slots per supergroup
GPC = 256
NSG = GPC // SG
NBLK = GPC // 128


def _derived(LFIX):
    assert LFIX % 64 == 0 and 512 < LFIX <= 1024
    LNM = ((LFIX + 127) // 128) * 128
    WPS = LNM // 128
    NPADF = GPC * LFIX
    NW = GPC * WPS
    return LNM, WPS, NPADF, NW


def _score_runs(LFIX):
    """Static (slot_local, runs) where each run maps a slot's node segment to
    thc coords: (half, col0, l0, ln). thc col = t*1024 + q*512 + (n%512),
    half = ((n%2048)//512) & 1, q = ((n%2048)//512) >> 1, t = n//2048."""
    out = []
    for j in range(SG):
        n0 = j * LFIX
        runs = []
        n = n0
        while n < n0 + LFIX:
            nxt = min((n // 512 + 1) * 512, n0 + LFIX)
            l0 = n - n0
            if l0 < 512 < (nxt - n0):
                nxt = n0 + 512
            b = (n % 2048) // 512
            half = b & 1
            col0 = (n // 2048) * 1024 + (b >> 1) * 512 + (n % 512)
            runs.append((half, col0, l0, nxt - n))
            n = nxt
        out.append(runs)
    return out


def build_nc_v2(reps=1, LFIX=LFIX0):
    LNM, WPS, NPADF, NW = _derived(LFIX)
    nc = bacc.Bacc(None, target_bir_lowering=False, debug=False)

    xT = nc.declare_dram_parameter("xT", [D, NPADF], BF16, isOutput=False)
    xs = nc.declare_dram_parameter("xs", [D, NW * D], FP8, isOutput=False)
    attW = nc.declare_dram_parameter("attW", [D, A], BF16, isOutput=False)
    attb2 = nc.declare_dram_parameter("attb2", [D, 1], F32, isOutput=False)
    ctxoh = nc.declare_dram_parameter("ctxoh", [D, 16 * 32], BF16,
                                      isOutput=False)
    ident8 = nc.declare_dram_parameter("ident8", [D, 8], BF16, isOutput=False)
    outW = nc.declare_dram_parameter("outW", [D, D], BF16, isOutput=False)
    outb = nc.declare_dram_parameter("outb", [D, 1], F32, isOutput=False)
    selW1 = nc.declare_dram_parameter("selW1", [3 * D, D], BF16,
                                      isOutput=False)
    selb1 = nc.declare_dram_parameter("selb1", [D, 1], F32, isOutput=False)
    selW2 = nc.declare_dram_parameter("selW2", [D, 3], BF16, isOutput=False)
    selb2 = nc.declare_dram_parameter("selb2", [3, 1], F32, isOutput=False)
    countsP = nc.declare_dram_parameter("counts", [GPC, 1], F32,
                                        isOutput=False)
    zcorrR = nc.declare_dram_parameter("zcorrR", [NSG * 128, 1], F32,
                                       isOutput=False)
    identP = nc.declare_dram_parameter("ident", [D, D], F32, isOutput=False)
    outP = nc.declare_dram_parameter("out", [GPC, D], F32, isOutput=True)

    runs_by_slot = _score_runs(LFIX)

    with tile_mod.TileContext(nc) as tc:
        with (
            tc.tile_pool(name="pools_ps", bufs=1, space="PSUM") as pools_ps,
            tc.tile_pool(name="const", bufs=1) as cpool,
        ):
            pools_bank = pools_ps.tile([D, 512], F32, name="pools_bank")

            attW_sb = cpool.tile([D, A], BF16)
            nc.sync.dma_start(attW_sb[:], attW[:])
            attb2_sb = cpool.tile([D, 1], F32)
            nc.sync.dma_start(attb2_sb[:], attb2[:])
            ctxoh_sb = cpool.tile([D, 16 * 32], BF16)
            nc.sync.dma_start(ctxoh_sb[:], ctxoh[:])
            ident8_sb = cpool.tile([D, 8], BF16)
            nc.sync.dma_start(ident8_sb[:], ident8[:])
            outW_sb = cpool.tile([D, D], BF16)
            nc.sync.dma_start(outW_sb[:], outW[:])
            outb_sb = cpool.tile([D, 1], F32)
            nc.sync.dma_start(outb_sb[:], outb[:])
            selW1_sb = cpool.tile([D, 3 * D], BF16)
            for k in range(3):
                nc.sync.dma_start(
                    selW1_sb[:, k * D:(k + 1) * D], selW1[k * D:(k + 1) * D, :]
                )
            selb1_sb = cpool.tile([D, 1], F32)
            nc.sync.dma_start(selb1_sb[:], selb1[:])
            selW2_sb = cpool.tile([D, 3], BF16)
            nc.sync.dma_start(selW2_sb[:], selW2[:])
            selb2_sb = cpool.tile([3, 1], F32)
            nc.sync.dma_start(selb2_sb[:], selb2[:])
            ident_sb = cpool.tile([D, D], F32)
            nc.sync.dma_start(ident_sb[:], identP[:])

            counts_bl, maxT = [], []
            for b in range(NBLK):
                cb = cpool.tile([128, 1], F32, name=f"counts_b{b}")
                nc.sync.dma_start(cb[:], countsP[b * 128:(b + 1) * 128, :])
                counts_bl.append(cb)
                maxT.append(cpool.tile([128, 128], F32, name=f"maxT{b}"))
            zcorrR_sb = []
            for g in range(NSG):
                zt_ = cpool.tile([128, 1], F32, name=f"zcorrR{g}")
                nc.sync.dma_start(zt_[:], zcorrR[g * 128:(g + 1) * 128, :])
                zcorrR_sb.append(zt_)

            with (
                tc.tile_pool(name="xf", bufs=3) as xfpool,
                tc.tile_pool(name="xn", bufs=2) as xnpool,
                tc.tile_pool(name="thc", bufs=2) as thcpool,
                tc.tile_pool(name="esb", bufs=2) as epool,
                tc.tile_pool(name="rhs", bufs=2) as rpool,
                tc.tile_pool(name="scr", bufs=3) as spool,
                tc.tile_pool(name="zt", bufs=2) as zpool,
                tc.tile_pool(name="thp", bufs=2, space="PSUM") as thps,
                tc.tile_pool(name="sps", bufs=1, space="PSUM") as sps_pool,
                tc.tile_pool(name="eps", bufs=1, space="PSUM") as eps_pool,
            ):
              for rep in range(reps):
                for sg in range(NSG):
                    xt = xfpool.tile([D, SG * LFIX], BF16, tag="xt",
                                     name=f"xt{sg}")
                    nc.sync.dma_start(
                        xt[:], xT[:, sg * SG * LFIX:(sg + 1) * SG * LFIX])
                    xs_t = xnpool.tile([D, SG * WPS * D], FP8, tag="xs",
                                       name=f"xs{sg}")
                    nc.sync.dma_start(
                        xs_t[:],
                        xs[:, sg * SG * WPS * D:(sg + 1) * SG * WPS * D])

                    # --- h + tanh (paired halves) --------------------------
                    thc = thcpool.tile([D, SG * LFIX // 2], BF16, tag="thc",
                                       name=f"thc{sg}")
                    for t in range(SG * LFIX // 2048):
                        thp = thps.tile([D, 1024], F32, tag="thp")
                        for q in (0, 1):
                            base = t * 2048 + q * 1024
                            nc.tensor.matmul(
                                thp[0:64, q * 512:q * 512 + 512],
                                attW_sb[:], xt[:, base:base + 512],
                                start=True, stop=True,
                                skip_group_check=True,
                            )
                            nc.tensor.matmul(
                                thp[64:128, q * 512:q * 512 + 512],
                                attW_sb[:], xt[:, base + 512:base + 1024],
                                start=True, stop=True,
                                skip_group_check=True,
                            )
                        nc.scalar.activation(
                            thc[:, t * 1024:(t + 1) * 1024], thp[:],
                            ACTF.Tanh, bias=attb2_sb[:], scale=1.0,
                        )

                    # --- scores (one-hot, col-tiled by subgroup) -----------
                    sps512 = sps_pool.tile([D, 512], F32, tag="sps512")
                    sps64 = sps_pool.tile([D, LFIX - 512], F32, tag="sps64")
                    seen = {}
                    last = {}
                    for j in range(SG):
                        c, r = j // 8, j % 8
                        for ri, (half, col0, l0, ln) in \
                                enumerate(runs_by_slot[j]):
                            tile512 = l0 < 512
                            last[(c, tile512)] = (j, ri)
                    for j in range(SG):
                        c, r = j // 8, j % 8
                        for ri, (half, col0, l0, ln) in \
                                enumerate(runs_by_slot[j]):
                            tile512 = l0 < 512
                            v = half * 8 + r
                            # full-K lhsT: inactive half rows are zero, so
                            # the other half's thc contributes nothing.
                            lhs = ctxoh_sb[:, v * 32:(v + 1) * 32]
                            dst = (sps512[32 * c:32 * c + 32, l0:l0 + ln]
                                   if tile512 else
                                   sps64[32 * c:32 * c + 32,
                                         l0 - 512:l0 - 512 + ln])
                            key = (c, tile512)
                            nc.tensor.matmul(
                                dst, lhs, thc[:, col0:col0 + ln],
                                start=key not in seen,
                                stop=last[key] == (j, ri),
                                tile_position=(0, 32 * c),
                                skip_group_check=True,
                            )
                            seen[key] = True

                    # --- exp + z ------------------------------------------
                    E_sb = epool.tile([D, LNM], BF16, tag="esb",
                                      name=f"E{sg}")
                    z32a = zpool.tile([D, 1], F32, tag="za")
                    z32b = zpool.tile([D, 1], F32, tag="zb")
                    nc.scalar.activation(E_sb[:, 0:512], sps512[:], ACTF.Exp,
                                         accum_out=z32a[:])
                    nc.scalar.activation(E_sb[:, 512:LFIX], sps64[:],
                                         ACTF.Exp, accum_out=z32b[:])
                    if LNM > LFIX:
                        nc.vector.memset(E_sb[:, LFIX:LNM], 0.0)
                    z32 = zpool.tile([D, 1], F32, tag="z")
                    nc.vector.tensor_add(z32[:], z32a[:], z32b[:])
                    # normalize E in place: w = 64 * e / (z - zcorr); the
                    # pools exsum then yields 64*attn_sum/z directly (the 64
                    # keeps fp8 w values out of denormals; undone in the
                    # final stage). zcorrR rows are permuted to z32's
                    # (c, r) row order on the host.
                    zc = zpool.tile([D, 1], F32, tag="zc")
                    nc.vector.tensor_sub(zc[:], z32[:],
                                         zcorrR_sb[sg][:])
                    rz = zpool.tile([D, 1], F32, tag="rz")
                    nc.vector.reciprocal(rz[:], zc[:])
                    nc.vector.tensor_scalar(E_sb[:], E_sb[:], rz[:], 64.0,
                                            ALU.mult, ALU.mult)

                    # --- E transpose -> rhs ones[0:160] | e[160:320] -------
                    # etile col layout (c, k, r): transpose (c, k) emits its
                    # 8 slot-rows as 8 consecutive cols.
                    NWSG = SG * WPS
                    # stage subgroup rows to partition base 0 (transpose-mode
                    # matmuls only work at row base 0 on HW)
                    ecopy = epool.tile([8, 4 * LNM], BF16, tag="ecopy")
                    for c in range(4):
                        nc.sync.dma_start(
                            ecopy[0:8, c * LNM:(c + 1) * LNM],
                            E_sb[32 * c:32 * c + 8, :])
                    etile = eps_pool.tile([D, NWSG], BF16, tag="et")
                    for c in range(4):
                        for k in range(WPS):
                            nc.tensor.transpose(
                                etile[:, (c * WPS + k) * 8:
                                      (c * WPS + k) * 8 + 8],
                                ecopy[0:8, c * LNM + 128 * k:
                                      c * LNM + 128 * k + 128],
                                ident8_sb[0:8, :],
                            )
                    rhs_t = rpool.tile([D, 2 * NWSG], FP8, tag="rhs")
                    nc.vector.memset(rhs_t[:, 0:NWSG], 1.0)
                    nc.vector.tensor_copy(rhs_t[:, NWSG:2 * NWSG], etile[:])

                    # --- pools matmuls ------------------------------------
                    # window (j=c*8+r, k): lhsT at xs col block j*WPS+k,
                    # e col idx = c*40 + k*8 + r; rhs = (ones[idx], e[idx]).
                    for j in range(SG):
                        c, r = j // 8, j % 8
                        s2 = (sg * SG + j) * 2
                        for k in range(WPS):
                            wi = j * WPS + k
                            idx = c * 8 * WPS + k * 8 + r
                            nc.tensor.matmul(
                                pools_bank[:, s2:s2 + 2],
                                xs_t[:, wi * D:(wi + 1) * D],
                                rhs_t[:, idx:idx + NWSG + 1:NWSG],
                                start=(k == 0), stop=(k == WPS - 1),
                            )

                    # --- max ----------------------------------------------
                    for j in range(SG):
                        g = sg * SG + j
                        b, col = g // 128, g % 128
                        scr = spool.tile([D, LFIX], BF16, tag="scr")
                        nc.vector.tensor_scalar(
                            scr[:], xt[:, j * LFIX:(j + 1) * LFIX], 1.0,
                            None, ALU.mult, ALU.max,
                            accum_out=maxT[b][:, col:col + 1],
                        )

            # ---- final stage --------------------------------------------
            with (
                tc.tile_pool(name="fin", bufs=1) as fpool,
                tc.tile_pool(name="fps", bufs=1, space="PSUM") as fps,
            ):
                for b in range(NBLK):
                    sumT = fpool.tile([128, 128], F32, name=f"sumT{b}")
                    nc.vector.tensor_copy(
                        sumT[:], pools_bank[:, b * 256:b * 256 + 256:2])
                    exT = fpool.tile([128, 128], F32, name=f"exT{b}")
                    nc.vector.tensor_copy(
                        exT[:], pools_bank[:, b * 256 + 1:b * 256 + 256:2])

                    rc = fpool.tile([128, 1], F32, name=f"rc{b}")
                    nc.vector.reciprocal(rc[:], counts_bl[b][:])

                    exbf = fpool.tile([128, 128], BF16, name=f"exbf{b}")
                    nc.vector.tensor_scalar(exbf[:], exT[:], 1.0 / 64.0,
                                            None, ALU.mult)
                    apT_ps = fps.tile([128, 128], F32, tag="apT")
                    nc.tensor.matmul(apT_ps[:], outW_sb[:], exbf[:],
                                     start=True, stop=True)
                    apT = fpool.tile([128, 128], F32, name=f"apT{b}")
                    nc.scalar.activation(apT[:], apT_ps[:], ACTF.Identity,
                                         bias=outb_sb[:], scale=1.0)

                    mean_gd = fpool.tile([128, 128], F32, name=f"mean_gd{b}")
                    tp = fps.tile([128, 128], F32, tag="tp")
                    nc.tensor.transpose(tp[:], sumT[:], ident_sb[:])
                    nc.scalar.activation(mean_gd[:], tp[:], ACTF.Identity,
                                         bias=0.0, scale=rc[:])
                    max_gd = fpool.tile([128, 128], F32, name=f"max_gd{b}")
                    tp2 = fps.tile([128, 128], F32, tag="tp")
                    nc.tensor.transpose(tp2[:], maxT[b][:], ident_sb[:])
                    nc.scalar.activation(max_gd[:], tp2[:], ACTF.Identity,
                                         bias=0.0, scale=1.0)
                    attn_gd = fpool.tile([128, 128], F32, name=f"attn_gd{b}")
                    tp3 = fps.tile([128, 128], F32, tag="tp")
                    nc.tensor.transpose(tp3[:], apT[:], ident_sb[:])
                    nc.scalar.activation(attn_gd[:], tp3[:], ACTF.Identity,
                                         bias=0.0, scale=1.0)

                    poolsT_bf = []
                    for nm, gd in (("m", mean_gd), ("x", max_gd),
                                   ("a", attn_gd)):
                        tpp = fps.tile([128, 128], F32, tag="tp")
                        nc.tensor.transpose(tpp[:], gd[:], ident_sb[:])
                        tbf = fpool.tile([128, 128], BF16, name=f"p{nm}T{b}")
                        nc.scalar.activation(tbf[:], tpp[:], ACTF.Identity,
                                             bias=0.0, scale=1.0)
                        poolsT_bf.append(tbf)

                    hid_ps = fps.tile([128, 128], F32, tag="hid")
                    for k in range(3):
                        nc.tensor.matmul(
                            hid_ps[:], selW1_sb[:, k * D:(k + 1) * D],
                            poolsT_bf[k][:],
                            start=(k == 0), stop=(k == 2),
                        )
                    hid_bf = fpool.tile([128, 128], BF16, name=f"hid{b}")
                    nc.scalar.activation(hid_bf[:], hid_ps[:], ACTF.Relu,
                                         bias=selb1_sb[:], scale=1.0)

                    lg_ps = fps.tile([3, 128], F32, tag="lg")
                    nc.tensor.matmul(lg_ps[:], selW2_sb[:], hid_bf[:],
                                     start=True, stop=True)
                    lgT = fpool.tile([3, 128], F32, name=f"lgT{b}")
                    nc.scalar.activation(lgT[:], lg_ps[:], ACTF.Identity,
                                         bias=selb2_sb[:], scale=1.0)

                    lg_ps2 = fps.tile([128, 3], F32, tag="lgt")
                    nc.tensor.transpose(lg_ps2[:], lgT[:], ident_sb[0:3, 0:3])
                    lg = fpool.tile([128, 3], F32, name=f"lg{b}")
                    nc.scalar.activation(lg[:], lg_ps2[:], ACTF.Identity,
                                         bias=0.0, scale=1.0)

                    m3 = fpool.tile([128, 1], F32, name=f"m3{b}")
                    nc.vector.tensor_reduce(m3[:], lg[:], mybir.AxisListType.X,
                                            ALU.max)
                    nm3 = fpool.tile([128, 1], F32, name=f"nm3{b}")
                    nc.vector.tensor_scalar(nm3[:], m3[:], -1.0, None,
                                            ALU.mult)
                    ew = fpool.tile([128, 3], F32, name=f"ew{b}")
                    den = fpool.tile([128, 1], F32, name=f"den{b}")
                    nc.scalar.activation(ew[:], lg[:], ACTF.Exp,
                                         bias=nm3[:], scale=1.0,
                                         accum_out=den[:])
                    rden = fpool.tile([128, 1], F32, name=f"rden{b}")
                    nc.vector.reciprocal(rden[:], den[:])
                    w3 = fpool.tile([128, 3], F32, name=f"w3{b}")
                    nc.vector.tensor_scalar(w3[:], ew[:], rden[:], None,
                                            ALU.mult)

                    t1 = fpool.tile([128, 128], F32, name=f"t1{b}")
                    nc.vector.tensor_scalar(t1[:], mean_gd[:], w3[:, 0:1],
                                            None, ALU.mult)
                    t2 = fpool.tile([128, 128], F32, name=f"t2{b}")
                    nc.vector.scalar_tensor_tensor(
                        out=t2[:], in0=max_gd[:], scalar=w3[:, 1:2], in1=t1[:],
                        op0=ALU.mult, op1=ALU.add,
                    )
                    out_sb = fpool.tile([128, D], F32, name=f"out_sb{b}")
                    nc.vector.scalar_tensor_tensor(
                        out=out_sb[:], in0=attn_gd[:],
                        scalar=w3[:, 2:3], in1=t2[:],
                        op0=ALU.mult, op1=ALU.add,
                    )
                    nc.sync.dma_start(
                        outP[b * 128:(b + 1) * 128, :], out_sb[:]
                    )

    nc.compile()
    return nc


# --------------------------------------------------------------------------
# host orchestration
# --------------------------------------------------------------------------

_CACHE = {}


def _shard_meta(batch, G):
    starts = np.searchsorted(batch, np.arange(G + 1))
    counts = np.diff(starts).astype(np.int64)
    return starts, counts


def _prep_core_inputs(x, batch, weights, core, starts, counts, LFIX=LFIX0):
    LNM, WPS, NPADF, NW = _derived(LFIX)
    g0 = core * GPC
    cst = starts[g0:g0 + GPC + 1]
    ccounts = counts[g0:g0 + GPC]
    n_core = cst[-1] - cst[0]
    assert ccounts.max() <= LFIX, ccounts.max()

    xs_nodes = x[cst[0]:cst[-1]]
    local_g = np.repeat(np.arange(GPC), ccounts)
    intra = np.arange(n_core) - np.repeat(cst[:-1] - cst[0], ccounts)

    xbf = xs_nodes.astype(ml_dtypes.bfloat16)

    # fm copy
    destf = local_g * LFIX + intra
    xpf = np.zeros((GPC * LFIX, D), dtype=ml_dtypes.bfloat16)
    xpf[destf] = xbf
    xTa = np.ascontiguousarray(xpf.T)

    # nm staged copy (640-pad)
    destn = local_g * LNM + intra
    xpn = np.zeros((GPC * LNM, D), dtype=FP8NP)
    xpn[destn] = xs_nodes.astype(FP8NP)
    xsa = np.ascontiguousarray(
        xpn.reshape(NW, 128, D).transpose(1, 0, 2).reshape(128, NW * D))

    (att_W, att_b, att_ctx, out_W, out_b,
     sel_W1, sel_b1, sel_W2, sel_b2) = weights

    ctxoh = np.zeros((D, 16 * 32), np.float32)
    for half in range(2):
        for r in range(8):
            v = half * 8 + r
            ctxoh[half * 64:(half + 1) * 64, v * 32 + r] = att_ctx

    s_pad = float(np.tanh(att_b.astype(np.float64)) @
                  att_ctx.astype(np.float64))
    e_pad = math.exp(s_pad)
    zcorr = ((LFIX - ccounts) * e_pad).astype(np.float32)
    # z-row order: slot sg*32 + c*8 + r sits at row 32c+r of supergroup sg
    zcorrR = np.zeros((NSG * 128, 1), np.float32)
    for g in range(NSG):
        for c in range(4):
            for r in range(8):
                zcorrR[g * 128 + 32 * c + r, 0] = zcorr[g * SG + c * 8 + r]

    return {
        "xT": xTa,
        "xs": xsa,
        "attW": att_W.astype(ml_dtypes.bfloat16),
        "attb2": np.concatenate([att_b, att_b]).astype(np.float32)
                  .reshape(D, 1),
        "ctxoh": ctxoh.astype(ml_dtypes.bfloat16),
        "ident8": np.tile(np.eye(8, dtype=np.float32), (16, 1))
                   .astype(ml_dtypes.bfloat16),
        "outW": out_W.astype(ml_dtypes.bfloat16),
        "outb": out_b.astype(np.float32).reshape(D, 1),
        "selW1": sel_W1.astype(ml_dtypes.bfloat16),
        "selb1": sel_b1.astype(np.float32).reshape(D, 1),
        "selW2": sel_W2.astype(ml_dtypes.bfloat16),
        "selb2": sel_b2.astype(np.float32).reshape(3, 1),
        "counts": ccounts.astype(np.float32).reshape(GPC, 1),
        "zcorrR": zcorrR,
        "ident": np.eye(D, dtype=np.float32),
    }


def _run(x, batch, att_W, att_b, att_ctx, out_W, out_b,
         sel_W1, sel_b1, sel_W2, sel_b2, num_graphs, **spmd_kwargs):
    x = np.asarray(x)
    batch = np.asarray(batch).astype(np.int64)
    G = int(num_graphs)
    assert G == 2048 and x.shape == (1000000, D)

    starts, counts = _shard_meta(batch, G)

    LFIX = LFIX0
    cmax = int(counts.max())
    while cmax > LFIX:
        LFIX += 64
    key = ("nc", LFIX)
    if key not in _CACHE:
        _CACHE[key] = build_nc_v2(LFIX=LFIX)
    nc = _CACHE[key]
    _CACHE["nc"] = nc

    weights = (np.asarray(att_W), np.asarray(att_b), np.asarray(att_ctx),
               np.asarray(out_W), np.asarray(out_b),
               np.asarray(sel_W1), np.asarray(sel_b1),
               np.asarray(sel_W2), np.asarray(sel_b2))

    in_maps = [
        _prep_core_inputs(x, batch, weights, c, starts, counts, LFIX=LFIX)
        for c in range(NCORES)
    ]

    res = run_bass_kernel_spmd(nc, in_maps, core_ids=list(range(NCORES)),
                               **spmd_kwargs)
    outs = [np.asarray(res.results[c]["out"], dtype=np.float32)
            for c in range(NCORES)]
    return np.concatenate(outs, axis=0), res


def kernel(**inputs):
    return _run(**inputs)[0]
